# revision 1
# baseline (speedup 1.0000x reference)
"""Causal self-attention with RoPE on 8 Trainium2 NeuronCores.

Strategy (tensor-parallel over heads, SPMD-uniform, collective-free):
  - 12 heads -> 8 cores x 2 head slots (4 slots get zero weights).
  - Per core: QKV projection for its 2 heads in [channel, seq] layout;
    RoPE via 3 DVE tensor-tensor ops with sign-folded cos/sin tables;
    causal flash-style attention with scores kept transposed
    (S^T[keys, queries]) so P tiles feed the AV matmul directly; softmax
    denominators via a ones-column in V; per-head normalization; partial
    output projection through the core's slice of w_proj columns.
  - QKV chunk J is interleaved with attention chunk J (causality means
    chunk J only attends keys < 512(J+1)), keeping ScalarE (exp) busy
    from the start.
  - Host sums the 8 partial [C, T] outputs (the "all-reduce") and
    transposes back.  All matmuls run in float32r (TF32-like).
"""

import sys

sys.path.insert(0, "/opt/trn_rl_repo")

import numpy as np

import concourse.bass as bass
import concourse.mybir as mybir
import concourse.tile as tile
from concourse import bacc, bass_utils
from concourse.masks import make_identity

FP32 = mybir.dt.float32
FP32R = mybir.dt.float32r
AF = mybir.ActivationFunctionType
ALU = mybir.AluOpType

T = 4096
C = 768
D = 64
N_HEAD = 12
N_CORES = 8
CHUNK = 512          # query chunk (matmul free dim)
NCHUNK = T // CHUNK  # 8
KT = 128             # key tile
ROPE_BASE = 10000.0

# core -> (head_slot_a, head_slot_b); None = zero slot
HEAD_MAP = [(0, 8), (1, 9), (2, 10), (3, 11),
            (4, None), (5, None), (6, None), (7, None)]

_PROG = None  # cached compiled program


def build_program():
    """Build + compile the per-core Bass program (identical on all cores)."""
    nc = bacc.Bacc("TRN2", target_bir_lowering=False, debug=False,
                   num_devices=N_CORES)

    xT_d = nc.dram_tensor("xT", [C, T], FP32R, kind="ExternalInput").ap()
    wqk_u_d = nc.dram_tensor("wqk_u", [C, 256], FP32R, kind="ExternalInput").ap()
    wqk_w_d = nc.dram_tensor("wqk_w", [C, 256], FP32R, kind="ExternalInput").ap()
    w_v_d = nc.dram_tensor("w_v", [C, 128], FP32R, kind="ExternalInput").ap()
    w_pT_d = nc.dram_tensor("w_projT", [128, C], FP32R, kind="ExternalInput").ap()
    cos_d = nc.dram_tensor("rope_cos", [128, T], FP32, kind="ExternalInput").ap()
    sin_d = nc.dram_tensor("rope_sin", [128, T], FP32, kind="ExternalInput").ap()
    mask_d = nc.dram_tensor("masks", [128, 4 * CHUNK], FP32R, kind="ExternalInput").ap()
    out_d = nc.dram_tensor("outT", [C, T], FP32, kind="ExternalOutput").ap()

    with tile.TileContext(nc) as tc:
        with (
            tc.tile_pool(name="persist", bufs=1) as pers,
            tc.tile_pool(name="xin", bufs=2) as xin,
            tc.tile_pool(name="tmp", bufs=3) as tmps,
            tc.tile_pool(name="ptile", bufs=6) as ptile,
            tc.tile_pool(name="ostage", bufs=3) as ostage,
            tc.tile_pool(name="small", bufs=4) as small,
            tc.tile_pool(name="psUW", bufs=2, space="PSUM") as psUW,  # qkv accumulators
            tc.tile_pool(name="psS", bufs=3, space="PSUM") as psS,    # score tiles
            tc.tile_pool(name="psY", bufs=1, space="PSUM") as psY,    # y accum
            tc.tile_pool(name="psX", bufs=1, space="PSUM") as psX,    # aux (pb/tr) + o
        ):
            # ---- persistent SBUF ----
            wqk_u = pers.tile([128, 6, 256], FP32R)
            wqk_w = pers.tile([128, 6, 256], FP32R)
            w_v = pers.tile([128, 6, 128], FP32R)
            w_pT = pers.tile([128, C], FP32R)

            mask_sb = pers.tile([128, 4 * CHUNK], FP32R)
            QT = pers.tile([128, T], FP32R)   # rows 0-63 head A, 64-127 head B
            KTt = pers.tile([128, T], FP32R)
            V = pers.tile([128, 32, 130], FP32R)  # [key%128, keytile, vA|1|vB|1]
            Y = pers.tile([128, T], FP32R)    # normalized attention out [ych, q]
            ones_sb = pers.tile([128, D], FP32R)
            ident = pers.tile([128, 128], FP32)
            make_identity(nc, ident[:])

            nc.sync.dma_start(wqk_u[:], wqk_u_d.rearrange("(o p) m -> p o m", p=128))
            nc.gpsimd.dma_start(w_v[:], w_v_d.rearrange("(o p) m -> p o m", p=128))
            nc.gpsimd.dma_start(w_pT[:], w_pT_d[:])
            nc.gpsimd.dma_start(mask_sb[:], mask_d[:])
            ones_f32 = pers.tile([128, D], FP32)
            nc.any.memset(ones_f32[:], 1.0)
            # HAM warm-up: keep PE busy during the initial input DMAs so the
            # clock gate reaches 8/8 before the first real matmuls (results
            # discarded; the tiny copy keeps DCE from dropping the chain)
            warm_ps = psS.tile([128, 128], FP32, tag="s")
            for i in range(16):
                nc.tensor.matmul(warm_ps[:], ident[:, 0:128], ident[:, 0:128],
                                 start=True, stop=True)
            warm_sb = small.tile([1, 8], FP32, tag="warm")
            nc.vector.tensor_copy(warm_sb[:], warm_ps[0:1, 0:8])
            nc.vector.tensor_copy(ones_sb[:], ones_f32[:])
            nc.vector.tensor_copy(V[:, :, 64], ones_sb[:, 0:32])
            nc.vector.tensor_copy(V[:, :, 129], ones_sb[:, 0:32])

            def emit_qkv(J):
                cols = slice(J * CHUNK, (J + 1) * CHUNK)
                xt = xin.tile([128, 6, CHUNK], FP32R)
                xT_r = xT_d.rearrange("(o p) n -> p o n", p=128)
                nc.sync.dma_start(xt[:, 0:3, :], xT_r[:, 0:3, cols])
                nc.sync.dma_start(xt[:, 3:6, :], xT_r[:, 3:6, cols])
                if J == 0:
                    nc.sync.dma_start(
                        wqk_w[:], wqk_w_d.rearrange("(o p) m -> p o m", p=128))
                cs_sb = tmps.tile([128, CHUNK], FP32, tag="cs")
                sn_sb = tmps.tile([128, CHUNK], FP32, tag="sn")
                nc.sync.dma_start(cs_sb[:], cos_d[:, cols])
                nc.sync.dma_start(sn_sb[:], sin_d[:, cols])
                for qk, tgt in ((0, QT), (1, KTt)):
                    u_ps = psUW.tile([128, CHUNK], FP32, tag="uw")
                    w_ps = psUW.tile([128, CHUNK], FP32, tag="uw")
                    wcol = slice(qk * 128, qk * 128 + 128)
                    for k in range(6):
                        nc.tensor.matmul(u_ps[:], wqk_u[:, k, wcol], xt[:, k, :],
                                         start=(k == 0), stop=(k == 5))
                    for k in range(6):
                        nc.tensor.matmul(w_ps[:], wqk_w[:, k, wcol], xt[:, k, :],
                                         start=(k == 0), stop=(k == 5))
                    tm = tmps.tile([128, CHUNK], FP32R, tag="ropetmp")
                    nc.vector.tensor_tensor(tgt[:, cols], u_ps[:], cs_sb[:], ALU.mult)
                    nc.vector.tensor_tensor(tm[:], w_ps[:], sn_sb[:], ALU.mult)
                    nc.vector.tensor_tensor(tgt[:, cols], tgt[:, cols], tm[:], ALU.add)
                # v in [ch, seq] (N=512 full-rate), then PE-transpose per key tile
                v_ps = psUW.tile([128, CHUNK], FP32, tag="uw")
                for k in range(6):
                    nc.tensor.matmul(v_ps[:], w_v[:, k, :], xt[:, k, :],
                                     start=(k == 0), stop=(k == 5))
                vT_sb = tmps.tile([128, CHUNK], FP32, tag="vt")
                nc.vector.tensor_copy(vT_sb[:], v_ps[:])
                for s in range(4):
                    kt_idx = 4 * J + s
                    tr_ps = psX.tile([128, 128], FP32, tag="aux")
                    nc.tensor.transpose(tr_ps[:], vT_sb[:, s * 128:(s + 1) * 128], ident)
                    nc.vector.tensor_copy(V[:, kt_idx, 0:64], tr_ps[:, 0:64])
                    nc.vector.tensor_copy(V[:, kt_idx, 65:129], tr_ps[:, 64:128])

            def emit_att(J):
                cols = slice(J * CHUNK, (J + 1) * CHUNK)
                nkt = 4 * J + 4
                for h in range(2):
                    hsl = slice(64 * h, 64 * h + 64)
                    vsl = slice(65 * h, 65 * h + 65)
                    y_ps = psY.tile([65, CHUNK], FP32, tag="y")
                    for t in range(nkt):
                        d = t - 4 * J
                        qlo = max(0, 128 * d)   # cols < qlo have no valid keys in tile t
                        sub = slice(qlo, CHUNK)
                        qsub = slice(J * CHUNK + qlo, (J + 1) * CHUNK)
                        s_ps = psS.tile([128, CHUNK], FP32, tag="s")
                        nc.tensor.matmul(
                            s_ps[:, sub], KTt[hsl, t * KT:(t + 1) * KT], QT[hsl, qsub],
                            start=True, stop=True)
                        p_sb = ptile.tile([128, CHUNK], FP32R, tag="p")
                        nc.scalar.activation(p_sb[:, sub], s_ps[:, sub], AF.Exp, scale=0.125)
                        if d >= 0:
                            # only qq in [qlo, qlo+128) straddles the causal
                            # boundary; columns beyond are fully valid
                            msub = slice(qlo, qlo + KT)
                            nc.vector.tensor_tensor(
                                p_sb[:, msub], p_sb[:, msub],
                                mask_sb[:, d * CHUNK + qlo:d * CHUNK + qlo + KT],
                                ALU.mult)
                        nc.tensor.matmul(
                            y_ps[:, sub], V[:, t, vsl], p_sb[:, sub],
                            start=(t == 0), stop=(t == nkt - 1))
                    rc = small.tile([1, CHUNK], FP32R, tag="rc")
                    with nc.allow_low_precision(reason="f32r recip for softmax denom"):
                        nc.vector.reciprocal(rc[0:1, :], y_ps[64:65, :])
                    rb = small.tile([64, CHUNK], FP32R, tag="rb")
                    nc.gpsimd.partition_broadcast(rb[:], rc[0:1, :])
                    nc.vector.tensor_tensor(Y[hsl, cols], y_ps[0:64, :], rb[:], ALU.mult)
                for m in range(6):
                    # last chunk: transposes are done, so borrow the aux slot
                    # to double-buffer the projection psum
                    otag = "aux" if (J == NCHUNK - 1 and m % 2) else "o"
                    o_ps = psX.tile([128, CHUNK], FP32, tag=otag)
                    nc.tensor.matmul(o_ps[:], w_pT[:, m * 128:(m + 1) * 128],
                                     Y[:, cols], start=True, stop=True)
                    o_sb = ostage.tile([128, CHUNK], FP32, tag="osb")
                    nc.vector.tensor_copy(o_sb[:], o_ps[:])
                    nc.sync.dma_start(out_d[m * 128:(m + 1) * 128, cols], o_sb[:])

            # attention J emitted right after its QKV; later QKV fills PE idle
            for J in range(NCHUNK):
                emit_qkv(J)
                emit_att(J)

    nc.compile()
    return nc


def _rope_tables():
    theta = 1.0 / (ROPE_BASE ** (np.arange(0, D, 2, dtype=np.float32) / D))  # [32]
    freqs = np.arange(T, dtype=np.float32)[None, :] * theta[:, None]  # [32, T]
    cos32 = np.cos(freqs).astype(np.float32)
    sin32 = np.sin(freqs).astype(np.float32)
    cos128 = np.tile(cos32, (4, 1))
    sin128 = np.concatenate([-sin32, sin32, -sin32, sin32], axis=0)
    return cos128, sin128


def _masks():
    m = np.zeros((128, 4 * CHUNK), dtype=np.float32)
    kk = np.arange(128)[:, None]
    qq = np.arange(CHUNK)[None, :]
    for d in range(4):
        m[:, d * CHUNK:(d + 1) * CHUNK] = (128 * d + kk <= qq).astype(np.float32)
    return m


def _swap_halves(w):
    # w: [rows multiple of 64, C]; swap 32-row halves within each 64 block
    r = w.reshape(-1, 2, 32, w.shape[-1])
    return r[:, ::-1].reshape(w.shape)


def make_in_maps(x, w_attn, w_proj):
    xT = np.ascontiguousarray(x.reshape(T, C).T)  # [C, T]
    cos128, sin128 = _rope_tables()
    masks = _masks()
    in_maps = []
    for c in range(N_CORES):
        qk_rows = []   # rows of w_attn for [qA, qB, kA, kB]
        v_rows = []    # [vA, vB]
        p_cols = []    # w_proj columns for [A(64), B(64)]
        sel = HEAD_MAP[c]
        for part_base in (0, C):  # q rows then k rows
            for h in sel:
                if h is None:
                    qk_rows.append(np.zeros((64, C), np.float32))
                else:
                    qk_rows.append(w_attn[part_base + 64 * h: part_base + 64 * h + 64])
        for h in sel:
            if h is None:
                v_rows.append(np.zeros((64, C), np.float32))
                p_cols.append(np.zeros((C, 64), np.float32))
            else:
                v_rows.append(w_attn[2 * C + 64 * h: 2 * C + 64 * h + 64])
                p_cols.append(w_proj[:, 64 * h: 64 * h + 64])
        qk = np.concatenate(qk_rows, axis=0)          # [256, C]
        wqk_u = np.ascontiguousarray(qk.T)            # [C, 256]
        wqk_w = np.ascontiguousarray(_swap_halves(qk).T)
        w_v = np.ascontiguousarray(np.concatenate(v_rows, axis=0).T)  # [C, 128]
        w_pT = np.ascontiguousarray(np.concatenate(p_cols, axis=1).T)  # [128, C]
        in_maps.append({
            "xT": xT, "wqk_u": wqk_u, "wqk_w": wqk_w, "w_v": w_v,
            "w_projT": w_pT, "rope_cos": cos128, "rope_sin": sin128,
            "masks": masks,
        })
    return in_maps


def kernel(x, w_attn, w_proj):
    global _PROG
    x = np.asarray(x, dtype=np.float32)
    w_attn = np.asarray(w_attn, dtype=np.float32)
    w_proj = np.asarray(w_proj, dtype=np.float32)
    if _PROG is None:
        _PROG = build_program()
    nc = _PROG
    in_maps = make_in_maps(x, w_attn, w_proj)
    res = bass_utils.run_bass_kernel_spmd(nc, in_maps, core_ids=list(range(N_CORES)))
    acc = np.zeros((C, T), dtype=np.float64)
    for c in range(N_CORES):
        acc += res.results[c]["outT"].astype(np.float64)
    return np.ascontiguousarray(acc.T.astype(np.float32)).reshape(1, T, C)


if __name__ == "__main__":
    rng = np.random.default_rng(0)
    x = rng.standard_normal((1, T, C)).astype(np.float32)
    wa = (rng.standard_normal((3 * C, C)) * 0.02).astype(np.float32)
    wp = (rng.standard_normal((C, C)) * 0.02).astype(np.float32)
    y = kernel(x, wa, wp)
    print("kernel out", y.shape, y.dtype, float(np.abs(y).max()))



# revision 11
# speedup vs baseline: 1.0302x; 1.0302x over previous
"""Causal self-attention with RoPE on 8 Trainium2 NeuronCores.

Strategy (tensor-parallel over heads, collective-free, load-balanced):
  - 12 heads -> 8 cores x 2 head slots.  Slot A: core c owns head c in
    full.  Slot B: the 4 remaining heads (8..11) are each SHARED by the
    core pair (c, c+4): the pair splits head 8+(c%4) by query chunks
    {2,6,7} vs {0,1,3,4,5} -- both sets are exactly half the causal
    triangle (72 key-tile units each), so all 8 cores do equal work.
    The split is realized with a runtime If/Else on partition_id (one
    compiled program, two specialized code paths).
  - Per core: QKV projection for its 2 heads in [channel, seq] layout
    (the slot-B head's Q/K/V rows ride along for free -- matmul cost is
    set by the moving dim); RoPE via 3 DVE tensor-tensor ops with
    sign-folded cos/sin tables; causal flash-style attention with scores
    kept transposed (S^T[keys, queries]) so P tiles feed the AV matmul
    directly; softmax denominators via a ones-column in V; per-head
    normalization; partial output projection through the core's slice of
    w_proj columns.  Slot-B rows of Y are zeroed once so skipped chunks
    contribute nothing to the projection.
  - Q/K/V/P tiles are bf16 (matmul-rate identical, but avoids the fp32r
    small-N penalty and halves SBUF); score accumulation stays fp32 in
    PSUM and the softmax denominator/normalization stays fp32.
  - Score tiles are computed in PAIRS into one 2-bank PSUM tile so each
    ScalarE exp instruction covers [128, 2*512], amortizing the fixed
    activation overhead; causal-boundary masking runs on GpSimd.
  - Host sums the 8 partial [C, T] outputs (the "all-reduce") and
    transposes back.  All fp32 matmuls run in float32r (TF32-like).
"""

import sys

sys.path.insert(0, "/opt/trn_rl_repo")

import numpy as np

import concourse.bass as bass
import concourse.mybir as mybir
import concourse.tile as tile
from concourse import bacc, bass_utils
from concourse.masks import make_identity

FP32 = mybir.dt.float32
FP32R = mybir.dt.float32r
BF16 = mybir.dt.bfloat16
AF = mybir.ActivationFunctionType
ALU = mybir.AluOpType

T = 4096
C = 768
D = 64
N_HEAD = 12
N_CORES = 8
CHUNK = 512          # query chunk (matmul free dim)
NCHUNK = T // CHUNK  # 8
KT = 128             # key tile
ROPE_BASE = 10000.0

# core -> (head_slot_a, head_slot_b); slot B shared by cores (c, c+4)
HEAD_MAP = [(c, 8 + c % 4) for c in range(N_CORES)]
# query chunks of the slot-B head processed by cores 0-3 / cores 4-7
# (both sets cover 72 causal key-tile units -- an exact 50/50 split)
BCHUNKS_LO = (2, 6, 7)
BCHUNKS_HI = (0, 1, 3, 4, 5)

_PROG = None  # cached compiled program


def build_program():
    """Build + compile the per-core Bass program (identical on all cores)."""
    nc = bacc.Bacc("TRN2", target_bir_lowering=False, debug=False,
                   num_devices=N_CORES)

    xT_d = nc.dram_tensor("xT", [C, T], FP32R, kind="ExternalInput").ap()
    wqk_u_d = nc.dram_tensor("wqk_u", [C, 256], FP32R, kind="ExternalInput").ap()
    wqk_w_d = nc.dram_tensor("wqk_w", [C, 256], FP32R, kind="ExternalInput").ap()
    w_v_d = nc.dram_tensor("w_v", [C, 128], FP32R, kind="ExternalInput").ap()
    w_pT_d = nc.dram_tensor("w_projT", [128, C], FP32R, kind="ExternalInput").ap()
    cos_d = nc.dram_tensor("rope_cos", [128, T], FP32, kind="ExternalInput").ap()
    sin_d = nc.dram_tensor("rope_sin", [128, T], FP32, kind="ExternalInput").ap()
    mask_d = nc.dram_tensor("masks", [128, 4 * CHUNK], BF16, kind="ExternalInput").ap()
    out_d = nc.dram_tensor("outT", [C, T], FP32, kind="ExternalOutput").ap()

    with tile.TileContext(nc) as tc:
        with (
            tc.tile_pool(name="persist", bufs=1) as pers,
            tc.tile_pool(name="xin", bufs=2) as xin,
            tc.tile_pool(name="tmp", bufs=3) as tmps,
            tc.tile_pool(name="ptile", bufs=6) as ptile,
            tc.tile_pool(name="ostage", bufs=3) as ostage,
            tc.tile_pool(name="small", bufs=4) as small,
            tc.tile_pool(name="psUW", bufs=2, space="PSUM") as psUW,  # qkv accum + v transposes
            tc.tile_pool(name="psS", bufs=2, space="PSUM") as psS,    # paired score tiles
            tc.tile_pool(name="psY", bufs=1, space="PSUM") as psY,    # y accum
            tc.tile_pool(name="psX", bufs=1, space="PSUM") as psX,    # warm + out proj
        ):
            # ---- persistent SBUF ----
            wqk_u = pers.tile([128, 6, 256], FP32R)
            wqk_w = pers.tile([128, 6, 256], FP32R)
            w_v = pers.tile([128, 6, 128], FP32R)
            w_pT = pers.tile([128, C], FP32R)

            mask_sb = pers.tile([128, 4 * CHUNK], BF16)
            QT = pers.tile([128, T], BF16)    # rows 0-63 head A, 64-127 head B
            KTt = pers.tile([128, T], BF16)
            V = pers.tile([128, 32, 130], BF16)  # [key%128, keytile, vA|1|vB|1]
            Y = pers.tile([128, T], FP32R)    # normalized attention out [ych, q]
            ones_sb = pers.tile([128, D], BF16)
            ident = pers.tile([128, 128], BF16)
            make_identity(nc, ident[:])

            nc.sync.dma_start(wqk_u[:], wqk_u_d.rearrange("(o p) m -> p o m", p=128))
            nc.gpsimd.dma_start(w_v[:], w_v_d.rearrange("(o p) m -> p o m", p=128))
            nc.gpsimd.dma_start(w_pT[:], w_pT_d[:])
            nc.gpsimd.dma_start(mask_sb[:], mask_d[:])
            ones_f32 = pers.tile([128, D], FP32)
            nc.any.memset(ones_f32[:], 1.0)
            # HAM warm-up: keep PE busy during the initial input DMAs so the
            # clock gate reaches 8/8 before the first real matmuls (results
            # discarded; the tiny copy keeps DCE from dropping the chain)
            warm_ps = psX.tile([128, 128], FP32, tag="o")
            for i in range(16):
                nc.tensor.matmul(warm_ps[:], ident[:, 0:128], ident[:, 0:128],
                                 start=True, stop=True)
            warm_sb = small.tile([1, 8], FP32, tag="warm")
            nc.vector.tensor_copy(warm_sb[:], warm_ps[0:1, 0:8])
            nc.vector.tensor_copy(ones_sb[:], ones_f32[:])
            nc.vector.tensor_copy(V[:, :, 64], ones_sb[:, 0:32])
            nc.vector.tensor_copy(V[:, :, 129], ones_sb[:, 0:32])
            # slot-B rows of Y feed the projection even for query chunks this
            # core skips (the partner core owns them) -- keep those zero
            # (walrus rejects memset with an f32r value type; bitcast to f32)
            nc.vector.memset(Y[64:128, :].bitcast(FP32), 0.0)

            def emit_qkv(J):
                cols = slice(J * CHUNK, (J + 1) * CHUNK)
                xt = xin.tile([128, 6, CHUNK], FP32R)
                xT_r = xT_d.rearrange("(o p) n -> p o n", p=128)
                nc.sync.dma_start(xt[:, 0:3, :], xT_r[:, 0:3, cols])
                nc.sync.dma_start(xt[:, 3:6, :], xT_r[:, 3:6, cols])
                if J == 0:
                    nc.sync.dma_start(
                        wqk_w[:], wqk_w_d.rearrange("(o p) m -> p o m", p=128))
                cs_sb = tmps.tile([128, CHUNK], FP32, tag="cs")
                sn_sb = tmps.tile([128, CHUNK], FP32, tag="sn")
                nc.sync.dma_start(cs_sb[:], cos_d[:, cols])
                nc.sync.dma_start(sn_sb[:], sin_d[:, cols])
                for qk, tgt in ((0, QT), (1, KTt)):
                    u_ps = psUW.tile([128, CHUNK], FP32, tag="uw")
                    w_ps = psUW.tile([128, CHUNK], FP32, tag="uw")
                    wcol = slice(qk * 128, qk * 128 + 128)
                    for k in range(6):
                        nc.tensor.matmul(u_ps[:], wqk_u[:, k, wcol], xt[:, k, :],
                                         start=(k == 0), stop=(k == 5))
                    for k in range(6):
                        nc.tensor.matmul(w_ps[:], wqk_w[:, k, wcol], xt[:, k, :],
                                         start=(k == 0), stop=(k == 5))
                    tm = tmps.tile([128, CHUNK], BF16, tag="ropetmp")
                    nc.vector.tensor_tensor(tgt[:, cols], u_ps[:], cs_sb[:], ALU.mult)
                    nc.vector.tensor_tensor(tm[:], w_ps[:], sn_sb[:], ALU.mult)
                    nc.vector.tensor_tensor(tgt[:, cols], tgt[:, cols], tm[:], ALU.add)
                # v in [ch, seq] (N=512 full-rate), then PE-transpose per key tile
                v_ps = psUW.tile([128, CHUNK], FP32, tag="uw")
                for k in range(6):
                    nc.tensor.matmul(v_ps[:], w_v[:, k, :], xt[:, k, :],
                                     start=(k == 0), stop=(k == 5))
                vT_sb = tmps.tile([128, CHUNK], BF16, tag="vt")
                nc.vector.tensor_copy(vT_sb[:], v_ps[:])
                for s in range(4):
                    kt_idx = 4 * J + s
                    tr_ps = psUW.tile([128, 128], BF16, tag="uw")
                    nc.tensor.transpose(tr_ps[:], vT_sb[:, s * 128:(s + 1) * 128], ident)
                    nc.vector.tensor_copy(V[:, kt_idx, 0:64], tr_ps[:, 0:64])
                    nc.vector.tensor_copy(V[:, kt_idx, 65:129], tr_ps[:, 64:128])

            def emit_att(J, heads):
                cols = slice(J * CHUNK, (J + 1) * CHUNK)
                nkt = 4 * J + 4
                for h in heads:
                    hsl = slice(64 * h, 64 * h + 64)
                    vsl = slice(65 * h, 65 * h + 65)
                    y_ps = psY.tile([65, CHUNK], FP32, tag="y")
                    for g in range(nkt // 2):
                        s_ps = psS.tile([128, 2, CHUNK], FP32, tag="s")
                        # both tiles of the pair compute from the pair's first
                        # valid query column so one exp instruction can cover
                        # the whole pair; the second tile's columns below its
                        # own causal start are computed-but-never-read
                        q0 = max(0, 128 * (2 * g - 4 * J))
                        for i in range(2):
                            t = 2 * g + i
                            qsub = slice(J * CHUNK + q0, (J + 1) * CHUNK)
                            nc.tensor.matmul(
                                s_ps[:, i, q0:CHUNK],
                                KTt[hsl, t * KT:(t + 1) * KT], QT[hsl, qsub],
                                start=True, stop=True)
                        p_sb = ptile.tile([128, 2, CHUNK], BF16, tag="p")
                        nc.scalar.activation(p_sb[:, :, q0:CHUNK], s_ps[:, :, q0:CHUNK],
                                             AF.Exp, scale=0.125)
                        for i in range(2):
                            t = 2 * g + i
                            d = t - 4 * J
                            qlo = max(0, 128 * d)
                            if d >= 0:
                                # only qq in [qlo, qlo+128) straddles the causal
                                # boundary; columns beyond are fully valid
                                msub = slice(qlo, qlo + KT)
                                nc.gpsimd.tensor_tensor(
                                    p_sb[:, i, msub], p_sb[:, i, msub],
                                    mask_sb[:, d * CHUNK + qlo:d * CHUNK + qlo + KT],
                                    ALU.mult)
                            sub = slice(qlo, CHUNK)
                            nc.tensor.matmul(
                                y_ps[:, sub], V[:, t, vsl], p_sb[:, i, sub],
                                start=(t == 0), stop=(t == nkt - 1))
                    rc = small.tile([1, CHUNK], FP32R, tag="rc")
                    with nc.allow_low_precision(reason="f32r recip for softmax denom"):
                        nc.vector.reciprocal(rc[0:1, :], y_ps[64:65, :])
                    rb = small.tile([64, CHUNK], FP32R, tag="rb")
                    nc.gpsimd.partition_broadcast(rb[:], rc[0:1, :])
                    nc.vector.tensor_tensor(Y[hsl, cols], y_ps[0:64, :], rb[:], ALU.mult)
                for m in range(6):
                    o_ps = psX.tile([128, CHUNK], FP32, tag="o")
                    nc.tensor.matmul(o_ps[:], w_pT[:, m * 128:(m + 1) * 128],
                                     Y[:, cols], start=True, stop=True)
                    o_sb = ostage.tile([128, CHUNK], FP32, tag="osb")
                    nc.vector.tensor_copy(o_sb[:], o_ps[:])
                    nc.sync.dma_start(out_d[m * 128:(m + 1) * 128, cols], o_sb[:])

            def emit_body(bchunks):
                # attention J emitted right after its QKV; later QKV fills
                # PE idle
                for J in range(NCHUNK):
                    emit_qkv(J)
                    emit_att(J, [0, 1] if J in bchunks else [0])

            pid = nc.partition_id()
            with tc.If(pid < 4) as cmp:
                emit_body(BCHUNKS_LO)
            with cmp.Else():
                emit_body(BCHUNKS_HI)

    nc.compile()
    return nc


def _rope_tables():
    theta = 1.0 / (ROPE_BASE ** (np.arange(0, D, 2, dtype=np.float32) / D))  # [32]
    freqs = np.arange(T, dtype=np.float32)[None, :] * theta[:, None]  # [32, T]
    cos32 = np.cos(freqs).astype(np.float32)
    sin32 = np.sin(freqs).astype(np.float32)
    cos128 = np.tile(cos32, (4, 1))
    sin128 = np.concatenate([-sin32, sin32, -sin32, sin32], axis=0)
    return cos128, sin128


def _masks():
    import ml_dtypes
    m = np.zeros((128, 4 * CHUNK), dtype=np.float32)
    kk = np.arange(128)[:, None]
    qq = np.arange(CHUNK)[None, :]
    for d in range(4):
        m[:, d * CHUNK:(d + 1) * CHUNK] = (128 * d + kk <= qq).astype(np.float32)
    return m.astype(ml_dtypes.bfloat16)


def _swap_halves(w):
    # w: [rows multiple of 64, C]; swap 32-row halves within each 64 block
    r = w.reshape(-1, 2, 32, w.shape[-1])
    return r[:, ::-1].reshape(w.shape)


def make_in_maps(x, w_attn, w_proj):
    xT = np.ascontiguousarray(x.reshape(T, C).T)  # [C, T]
    cos128, sin128 = _rope_tables()
    masks = _masks()
    in_maps = []
    for c in range(N_CORES):
        qk_rows = []   # rows of w_attn for [qA, qB, kA, kB]
        v_rows = []    # [vA, vB]
        p_cols = []    # w_proj columns for [A(64), B(64)]
        sel = HEAD_MAP[c]
        for part_base in (0, C):  # q rows then k rows
            for h in sel:
                if h is None:
                    qk_rows.append(np.zeros((64, C), np.float32))
                else:
                    qk_rows.append(w_attn[part_base + 64 * h: part_base + 64 * h + 64])
        for h in sel:
            if h is None:
                v_rows.append(np.zeros((64, C), np.float32))
                p_cols.append(np.zeros((C, 64), np.float32))
            else:
                v_rows.append(w_attn[2 * C + 64 * h: 2 * C + 64 * h + 64])
                p_cols.append(w_proj[:, 64 * h: 64 * h + 64])
        qk = np.concatenate(qk_rows, axis=0)          # [256, C]
        wqk_u = np.ascontiguousarray(qk.T)            # [C, 256]
        wqk_w = np.ascontiguousarray(_swap_halves(qk).T)
        w_v = np.ascontiguousarray(np.concatenate(v_rows, axis=0).T)  # [C, 128]
        w_pT = np.ascontiguousarray(np.concatenate(p_cols, axis=1).T)  # [128, C]
        in_maps.append({
            "xT": xT, "wqk_u": wqk_u, "wqk_w": wqk_w, "w_v": w_v,
            "w_projT": w_pT, "rope_cos": cos128, "rope_sin": sin128,
            "masks": masks,
        })
    return in_maps


def kernel(x, w_attn, w_proj):
    global _PROG
    x = np.asarray(x, dtype=np.float32)
    w_attn = np.asarray(w_attn, dtype=np.float32)
    w_proj = np.asarray(w_proj, dtype=np.float32)
    if _PROG is None:
        _PROG = build_program()
    nc = _PROG
    in_maps = make_in_maps(x, w_attn, w_proj)
    res = bass_utils.run_bass_kernel_spmd(nc, in_maps, core_ids=list(range(N_CORES)))
    acc = np.zeros((C, T), dtype=np.float64)
    for c in range(N_CORES):
        acc += res.results[c]["outT"].astype(np.float64)
    return np.ascontiguousarray(acc.T.astype(np.float32)).reshape(1, T, C)


if __name__ == "__main__":
    rng = np.random.default_rng(0)
    x = rng.standard_normal((1, T, C)).astype(np.float32)
    wa = (rng.standard_normal((3 * C, C)) * 0.02).astype(np.float32)
    wp = (rng.standard_normal((C, C)) * 0.02).astype(np.float32)
    y = kernel(x, wa, wp)
    print("kernel out", y.shape, y.dtype, float(np.abs(y).max()))


# revision 47
# speedup vs baseline: 3.7314x; 3.6222x over previous
"""Causal self-attention with RoPE on 8 Trainium2 NeuronCores.

Strategy (tensor-parallel over heads, collective-free, load-balanced):
  - 12 heads -> 8 cores x 2 head slots.  Slot A: core c owns head c in
    full.  Slot B: the 4 remaining heads (8..11) are each SHARED by the
    core pair (c, c+4): the pair splits head 8+(c%4) by query chunks
    {2,6,7} vs {0,1,3,4,5} -- both sets are exactly half the causal
    triangle (72 key-tile units each), so all 8 cores do equal work.
    The split is realized with a runtime If/Else on partition_id (one
    compiled program, two specialized code paths).
  - Per core: QKV projection for its 2 heads in [channel, seq] layout
    (the slot-B head's Q/K/V rows ride along for free -- matmul cost is
    set by the moving dim); RoPE via 3 DVE tensor-tensor ops with
    sign-folded cos/sin tables; causal flash-style attention with scores
    kept transposed (S^T[keys, queries]) so P tiles feed the AV matmul
    directly; softmax denominators via a ones-column in V; per-head
    normalization; partial output projection through the core's slice of
    w_proj columns.  Slot-B rows of Y are zeroed once so skipped chunks
    contribute nothing to the projection.
  - Q/K/V/P tiles are bf16 (matmul-rate identical, but avoids the fp32r
    small-N penalty and halves SBUF); score accumulation stays fp32 in
    PSUM and the softmax denominator/normalization stays fp32.
  - Score tiles are computed in PAIRS into one 2-bank PSUM tile so each
    ScalarE exp instruction covers [128, 2*512], amortizing the fixed
    activation overhead; causal-boundary masking runs on GpSimd.
  - Host sums the 8 partial [C, T] outputs (the "all-reduce") and
    transposes back.  All fp32 matmuls run in float32r (TF32-like).
"""

import sys

sys.path.insert(0, "/opt/trn_rl_repo")

import numpy as np

import concourse.bass as bass
import concourse.mybir as mybir
import concourse.tile as tile
from concourse import bacc, bass_utils
from concourse.masks import make_identity

FP32 = mybir.dt.float32
FP32R = mybir.dt.float32r
BF16 = mybir.dt.bfloat16
FP16 = mybir.dt.float16
AF = mybir.ActivationFunctionType
ALU = mybir.AluOpType

T = 4096
C = 768
D = 64
N_HEAD = 12
N_CORES = 8
CHUNK = 512          # query chunk (matmul free dim)
NCHUNK = T // CHUNK  # 8
KT = 128             # key tile
ROPE_BASE = 10000.0

# core -> (head_slot_a, head_slot_b); slot B shared by cores (c, c+4)
HEAD_MAP = [(c, 8 + c % 4) for c in range(N_CORES)]
# query chunks of the slot-B head processed by cores 0-3 / cores 4-7
# (both sets cover 72 causal key-tile units -- an exact 50/50 split)
BCHUNKS_LO = (2, 6, 7)
BCHUNKS_HI = (0, 1, 3, 4, 5)

_PROG = None  # cached compiled program


def build_program():
    """Build + compile the per-core Bass program (identical on all cores)."""
    nc = bacc.Bacc("TRN2", target_bir_lowering=False, debug=False,
                   num_devices=N_CORES)

    xT_d = nc.dram_tensor("xT", [C, T], BF16, kind="ExternalInput").ap()
    wqk_u_d = nc.dram_tensor("wqk_u", [C, 256], BF16, kind="ExternalInput").ap()
    wqk_w_d = nc.dram_tensor("wqk_w", [C, 256], BF16, kind="ExternalInput").ap()
    w_v_d = nc.dram_tensor("w_v", [C, 128], BF16, kind="ExternalInput").ap()
    w_pT_d = nc.dram_tensor("w_projT", [128, C], FP32R, kind="ExternalInput").ap()
    cos_d = nc.dram_tensor("rope_cos", [128, T], FP16, kind="ExternalInput").ap()
    sin_d = nc.dram_tensor("rope_sin", [128, T], FP16, kind="ExternalInput").ap()
    mask_d = nc.dram_tensor("masks", [128, 4 * CHUNK], BF16, kind="ExternalInput").ap()
    out_d = nc.dram_tensor("outT", [C, T], BF16, kind="ExternalOutput").ap()

    with tile.TileContext(nc) as tc:
        with (
            tc.tile_pool(name="persist", bufs=1) as pers,
            tc.tile_pool(name="xin", bufs=2) as xin,
            tc.tile_pool(name="tmp", bufs=3) as tmps,
            tc.tile_pool(name="ptile", bufs=6) as ptile,
            tc.tile_pool(name="ostage", bufs=3) as ostage,
            tc.tile_pool(name="small", bufs=4) as small,
            tc.tile_pool(name="psUW", bufs=2, space="PSUM") as psUW,  # qkv accum + v transposes
            tc.tile_pool(name="psS", bufs=2, space="PSUM") as psS,    # paired score tiles
            tc.tile_pool(name="psY", bufs=1, space="PSUM") as psY,    # y accum
            tc.tile_pool(name="psX", bufs=1, space="PSUM") as psX,    # warm + out proj
        ):
            # ---- persistent SBUF ----
            wqk_u = pers.tile([128, 6, 256], BF16)
            wqk_w = pers.tile([128, 6, 256], BF16)
            w_v = pers.tile([128, 6, 128], BF16)
            w_pT = pers.tile([128, C], FP32R)

            mask_sb = pers.tile([128, 4 * CHUNK], BF16)
            QT = pers.tile([128, T], BF16)    # rows 0-63 head A, 64-127 head B
            KTt = pers.tile([128, T], BF16)
            V = pers.tile([128, 32, 130], BF16)  # [key%128, keytile, vA|1|vB|1]
            Y = pers.tile([128, T], FP32R)    # normalized attention out [ych, q]
            ones_sb = pers.tile([128, D], BF16)
            ident = pers.tile([128, 128], BF16)
            make_identity(nc, ident[:])

            # only wqk_u is on the first-matmul critical path; the other
            # weight loads are emitted inside emit_qkv(0) after the first
            # x-chunk DMA so they don't delay it
            nc.sync.dma_start(wqk_u[:], wqk_u_d.rearrange("(o p) m -> p o m", p=128))
            ones_f32 = pers.tile([128, D], FP32)
            nc.any.memset(ones_f32[:], 1.0)
            # HAM warm-up: keep PE busy during the initial input DMAs so the
            # clock gate reaches 8/8 before the first real matmuls (results
            # discarded; the tiny copy keeps DCE from dropping the chain)
            warm_ps = psX.tile([128, 128], FP32, tag="o")
            for i in range(16):
                nc.tensor.matmul(warm_ps[:], ident[:, 0:128], ident[:, 0:128],
                                 start=True, stop=True)
            warm_sb = small.tile([1, 8], FP32, tag="warm")
            nc.vector.tensor_copy(warm_sb[:], warm_ps[0:1, 0:8])
            # Pool for the small setup copies -- DVE's early queue is the
            # chunk-0 RoPE critical path
            nc.gpsimd.tensor_copy(ones_sb[:], ones_f32[:])
            nc.gpsimd.tensor_copy(V[:, :, 64], ones_sb[:, 0:32])
            nc.gpsimd.tensor_copy(V[:, :, 129], ones_sb[:, 0:32])
            # dummy Pool work before make_identity's affine_select: delays the
            # HAM warm-up to land flush against the first QKV matmuls, keeping
            # the PE power-state ramp inside the initial DMA window
            nc.gpsimd.memset(Y[64:128, :].bitcast(FP32), 0.0)


            def emit_qkv(J):
                cols = slice(J * CHUNK, (J + 1) * CHUNK)
                xt = xin.tile([128, 6, CHUNK], BF16)
                xT_r = xT_d.rearrange("(o p) n -> p o n", p=128)
                nc.sync.dma_start(xt[:, 0:3, :], xT_r[:, 0:3, cols])
                nc.sync.dma_start(xt[:, 3:6, :], xT_r[:, 3:6, cols])
                cs_sb = tmps.tile([128, CHUNK], FP16, tag="cs")
                sn_sb = tmps.tile([128, CHUNK], FP16, tag="sn")
                nc.sync.dma_start(cs_sb[:], cos_d[:, cols])
                nc.sync.dma_start(sn_sb[:], sin_d[:, cols])
                if J == 0:
                    nc.sync.dma_start(
                        wqk_w[:], wqk_w_d.rearrange("(o p) m -> p o m", p=128))
                    nc.gpsimd.dma_start(
                        w_v[:], w_v_d.rearrange("(o p) m -> p o m", p=128))
                    nc.gpsimd.dma_start(mask_sb[:], mask_d[:])
                    nc.gpsimd.dma_start(w_pT[:], w_pT_d[:])

                for qk, tgt in ((0, QT), (1, KTt)):
                    u_ps = psUW.tile([128, CHUNK], FP32, tag="uw")
                    w_ps = psUW.tile([128, CHUNK], FP32, tag="uw")
                    wcol = slice(qk * 128, qk * 128 + 128)
                    for k in range(6):
                        nc.tensor.matmul(u_ps[:], wqk_u[:, k, wcol], xt[:, k, :],
                                         start=(k == 0), stop=(k == 5))
                    for k in range(6):
                        nc.tensor.matmul(w_ps[:], wqk_w[:, k, wcol], xt[:, k, :],
                                         start=(k == 0), stop=(k == 5))
                    tm = tmps.tile([128, CHUNK], BF16, tag="ropetmp")
                    nc.vector.tensor_tensor(tgt[:, cols], u_ps[:], cs_sb[:], ALU.mult)
                    nc.vector.tensor_tensor(tm[:], w_ps[:], sn_sb[:], ALU.mult)
                    nc.vector.tensor_tensor(tgt[:, cols], tgt[:, cols], tm[:], ALU.add)
                # v in [ch, seq] (N=512 full-rate), then PE-transpose per key tile
                v_ps = psUW.tile([128, CHUNK], FP32, tag="uw")
                for k in range(6):
                    nc.tensor.matmul(v_ps[:], w_v[:, k, :], xt[:, k, :],
                                     start=(k == 0), stop=(k == 5))
                vT_sb = tmps.tile([128, CHUNK], BF16, tag="vt")
                nc.vector.tensor_copy(vT_sb[:], v_ps[:])
                for s in range(4):
                    kt_idx = 4 * J + s
                    tr_ps = psUW.tile([128, 128], BF16, tag="uw")
                    nc.tensor.transpose(tr_ps[:], vT_sb[:, s * 128:(s + 1) * 128], ident)
                    nc.vector.tensor_copy(V[:, kt_idx, 0:64], tr_ps[:, 0:64])
                    nc.vector.tensor_copy(V[:, kt_idx, 65:129], tr_ps[:, 64:128])

            def emit_att(J, heads):
                cols = slice(J * CHUNK, (J + 1) * CHUNK)
                nkt = 4 * J + 4
                for h in heads:
                    hsl = slice(64 * h, 64 * h + 64)
                    vsl = slice(65 * h, 65 * h + 65)
                    y_ps = psY.tile([65, CHUNK], FP32, tag="y")
                    for g in range(nkt // 2):
                        s_ps = psS.tile([128, 2, CHUNK], FP32, tag="s")
                        # both tiles of the pair compute from the pair's first
                        # valid query column so one exp instruction can cover
                        # the whole pair; the second tile's columns below its
                        # own causal start are computed-but-never-read
                        q0 = max(0, 128 * (2 * g - 4 * J))
                        for i in range(2):
                            t = 2 * g + i
                            qsub = slice(J * CHUNK + q0, (J + 1) * CHUNK)
                            nc.tensor.matmul(
                                s_ps[:, i, q0:CHUNK],
                                KTt[hsl, t * KT:(t + 1) * KT], QT[hsl, qsub],
                                start=True, stop=True)
                        p_sb = ptile.tile([128, 2, CHUNK], BF16, tag="p")
                        nc.scalar.activation(p_sb[:, :, q0:CHUNK], s_ps[:, :, q0:CHUNK],
                                             AF.Exp, scale=0.125)
                        for i in range(2):
                            t = 2 * g + i
                            d = t - 4 * J
                            qlo = max(0, 128 * d)
                            if d >= 0:
                                # only qq in [qlo, qlo+128) straddles the causal
                                # boundary; columns beyond are fully valid
                                msub = slice(qlo, qlo + KT)
                                nc.gpsimd.tensor_tensor(
                                    p_sb[:, i, msub], p_sb[:, i, msub],
                                    mask_sb[:, d * CHUNK + qlo:d * CHUNK + qlo + KT],
                                    ALU.mult)
                            sub = slice(qlo, CHUNK)
                            nc.tensor.matmul(
                                y_ps[:, sub], V[:, t, vsl], p_sb[:, i, sub],
                                start=(t == 0), stop=(t == nkt - 1))
                    rc = small.tile([1, CHUNK], FP32R, tag="rc")
                    with nc.allow_low_precision(reason="f32r recip for softmax denom"):
                        nc.vector.reciprocal(rc[0:1, :], y_ps[64:65, :])
                    rb = small.tile([64, CHUNK], FP32R, tag="rb")
                    nc.gpsimd.partition_broadcast(rb[:], rc[0:1, :])
                    nc.vector.tensor_tensor(Y[hsl, cols], y_ps[0:64, :], rb[:], ALU.mult)
                # chunks where this core skips the slot-B head contract over
                # head A's 64 Y-rows only (the partner core owns the rest);
                # matmul cost is set by the moving dim, so this is free
                ymax = 128 if 1 in heads else 64
                for m in range(6):
                    # last chunk: QKV is done, so the psUW slots are free --
                    # borrow them to triple-buffer the projection tail
                    otag = "uw" if J == NCHUNK - 1 and m % 2 else "o"
                    o_ps = (psUW if otag == "uw" else psX).tile(
                        [128, CHUNK], FP32, tag=otag)
                    nc.tensor.matmul(o_ps[:], w_pT[0:ymax, m * 128:(m + 1) * 128],
                                     Y[0:ymax, cols], start=True, stop=True)
                    o_sb = ostage.tile([128, CHUNK], BF16, tag="osb")
                    if J == NCHUNK - 1 and m % 2:
                        # ScalarE is out of exp work by the last chunk's
                        # projection -- split the staging copies across engines
                        nc.scalar.copy(o_sb[:], o_ps[:])
                    else:
                        nc.vector.tensor_copy(o_sb[:], o_ps[:])
                    nc.sync.dma_start(out_d[m * 128:(m + 1) * 128, cols], o_sb[:])

            def emit_body(bchunks):
                # attention J emitted right after its QKV; later QKV fills
                # PE idle
                for J in range(NCHUNK):
                    emit_qkv(J)
                    emit_att(J, [0, 1] if J in bchunks else [0])

            pid = nc.partition_id()
            with tc.If(pid < 4) as cmp:
                emit_body(BCHUNKS_LO)
            with cmp.Else():
                emit_body(BCHUNKS_HI)

    nc.compile()
    return nc


def _rope_tables():
    theta = 1.0 / (ROPE_BASE ** (np.arange(0, D, 2, dtype=np.float32) / D))  # [32]
    freqs = np.arange(T, dtype=np.float32)[None, :] * theta[:, None]  # [32, T]
    cos32 = np.cos(freqs).astype(np.float32)
    sin32 = np.sin(freqs).astype(np.float32)
    cos128 = np.tile(cos32, (4, 1)).astype(np.float16)
    sin128 = np.concatenate([-sin32, sin32, -sin32, sin32], axis=0).astype(np.float16)
    return cos128, sin128


def _masks():
    import ml_dtypes
    m = np.zeros((128, 4 * CHUNK), dtype=np.float32)
    kk = np.arange(128)[:, None]
    qq = np.arange(CHUNK)[None, :]
    for d in range(4):
        m[:, d * CHUNK:(d + 1) * CHUNK] = (128 * d + kk <= qq).astype(np.float32)
    return m.astype(ml_dtypes.bfloat16)


def _swap_halves(w):
    # w: [rows multiple of 64, C]; swap 32-row halves within each 64 block
    r = w.reshape(-1, 2, 32, w.shape[-1])
    return r[:, ::-1].reshape(w.shape)


def make_in_maps(x, w_attn, w_proj):
    import ml_dtypes
    xT = np.ascontiguousarray(x.reshape(T, C).T).astype(ml_dtypes.bfloat16)  # [C, T]
    cos128, sin128 = _rope_tables()
    masks = _masks()
    in_maps = []
    for c in range(N_CORES):
        qk_rows = []   # rows of w_attn for [qA, qB, kA, kB]
        v_rows = []    # [vA, vB]
        p_cols = []    # w_proj columns for [A(64), B(64)]
        sel = HEAD_MAP[c]
        for part_base in (0, C):  # q rows then k rows
            for h in sel:
                if h is None:
                    qk_rows.append(np.zeros((64, C), np.float32))
                else:
                    qk_rows.append(w_attn[part_base + 64 * h: part_base + 64 * h + 64])
        for h in sel:
            if h is None:
                v_rows.append(np.zeros((64, C), np.float32))
                p_cols.append(np.zeros((C, 64), np.float32))
            else:
                v_rows.append(w_attn[2 * C + 64 * h: 2 * C + 64 * h + 64])
                p_cols.append(w_proj[:, 64 * h: 64 * h + 64])
        qk = np.concatenate(qk_rows, axis=0)          # [256, C]
        wqk_u = np.ascontiguousarray(qk.T).astype(ml_dtypes.bfloat16)  # [C, 256]
        wqk_w = np.ascontiguousarray(_swap_halves(qk).T).astype(ml_dtypes.bfloat16)
        w_v = np.ascontiguousarray(
            np.concatenate(v_rows, axis=0).T).astype(ml_dtypes.bfloat16)  # [C, 128]
        w_pT = np.ascontiguousarray(np.concatenate(p_cols, axis=1).T)  # [128, C]
        in_maps.append({
            "xT": xT, "wqk_u": wqk_u, "wqk_w": wqk_w, "w_v": w_v,
            "w_projT": w_pT, "rope_cos": cos128, "rope_sin": sin128,
            "masks": masks,
        })
    return in_maps


def kernel(x, w_attn, w_proj):
    global _PROG
    x = np.asarray(x, dtype=np.float32)
    w_attn = np.asarray(w_attn, dtype=np.float32)
    w_proj = np.asarray(w_proj, dtype=np.float32)
    if _PROG is None:
        _PROG = build_program()
    nc = _PROG
    in_maps = make_in_maps(x, w_attn, w_proj)
    res = bass_utils.run_bass_kernel_spmd(nc, in_maps, core_ids=list(range(N_CORES)))
    acc = np.zeros((C, T), dtype=np.float64)
    for c in range(N_CORES):
        acc += res.results[c]["outT"].astype(np.float64)
    return np.ascontiguousarray(acc.T.astype(np.float32)).reshape(1, T, C)


if __name__ == "__main__":
    rng = np.random.default_rng(0)
    x = rng.standard_normal((1, T, C)).astype(np.float32)
    wa = (rng.standard_normal((3 * C, C)) * 0.02).astype(np.float32)
    wp = (rng.standard_normal((C, C)) * 0.02).astype(np.float32)
    y = kernel(x, wa, wp)
    print("kernel out", y.shape, y.dtype, float(np.abs(y).max()))


# revision 53
# speedup vs baseline: 3.9124x; 1.0485x over previous
"""Causal self-attention with RoPE on 8 Trainium2 NeuronCores.

Strategy (tensor-parallel over heads, collective-free, load-balanced):
  - 12 heads -> 8 cores x 2 head slots.  Slot A: core c owns head c in
    full.  Slot B: the 4 remaining heads (8..11) are each SHARED by the
    core pair (c, c+4): the pair splits head 8+(c%4) by query chunks
    {2,5,7} vs {0,1,3,4,6} -- 68/76 causal key-tile units.  The slight
    static imbalance compensates cores 0-3's structural tail (their last
    chunk runs both heads' ScalarE-bound attention with no QKV left to
    overlap), making both variants finish together.  The split is
    realized with a runtime If/Else on partition_id (one compiled
    program, two specialized code paths).
  - Per core: QKV projection for its 2 heads in [channel, seq] layout
    (the slot-B head's Q/K/V rows ride along for free -- matmul cost is
    set by the moving dim); RoPE via 3 DVE tensor-tensor ops with
    sign-folded cos/sin tables; causal flash-style attention with scores
    kept transposed (S^T[keys, queries]) so P tiles feed the AV matmul
    directly; softmax denominators via a ones-column in V; per-head
    normalization; partial output projection through the core's slice of
    w_proj columns.  Slot-B rows of Y are zeroed once so skipped chunks
    contribute nothing to the projection.
  - Q/K/V/P tiles are bf16 (matmul-rate identical, but avoids the fp32r
    small-N penalty and halves SBUF); score accumulation stays fp32 in
    PSUM and the softmax denominator/normalization stays fp32.
  - Score tiles are computed in PAIRS into one 2-bank PSUM tile so each
    ScalarE exp instruction covers [128, 2*512], amortizing the fixed
    activation overhead; causal-boundary masking runs on GpSimd.
  - Host sums the 8 partial [C, T] outputs (the "all-reduce") and
    transposes back.  All fp32 matmuls run in float32r (TF32-like).
"""

import sys

sys.path.insert(0, "/opt/trn_rl_repo")

import numpy as np

import concourse.bass as bass
import concourse.mybir as mybir
import concourse.tile as tile
from concourse import bacc, bass_utils
from concourse.masks import make_identity

FP32 = mybir.dt.float32
FP32R = mybir.dt.float32r
BF16 = mybir.dt.bfloat16
FP16 = mybir.dt.float16
AF = mybir.ActivationFunctionType
ALU = mybir.AluOpType

T = 4096
C = 768
D = 64
N_HEAD = 12
N_CORES = 8
CHUNK = 512          # query chunk (matmul free dim)
NCHUNK = T // CHUNK  # 8
KT = 128             # key tile
ROPE_BASE = 10000.0

# core -> (head_slot_a, head_slot_b); slot B shared by cores (c, c+4)
HEAD_MAP = [(c, 8 + c % 4) for c in range(N_CORES)]
# query chunks of the slot-B head processed by cores 0-3 / cores 4-7
# (68/76 causal key-tile units: slightly lighter on cores 0-3, whose last
# chunk carries both heads' attention with no QKV left to overlap it)
BCHUNKS_LO = (2, 5, 7)
BCHUNKS_HI = (0, 1, 3, 4, 6)

_PROG = None  # cached compiled program


def build_program():
    """Build + compile the per-core Bass program (identical on all cores)."""
    nc = bacc.Bacc("TRN2", target_bir_lowering=False, debug=False,
                   num_devices=N_CORES)

    xT_d = nc.dram_tensor("xT", [C, T], BF16, kind="ExternalInput").ap()
    wqk_u_d = nc.dram_tensor("wqk_u", [C, 256], BF16, kind="ExternalInput").ap()
    wqk_w_d = nc.dram_tensor("wqk_w", [C, 256], BF16, kind="ExternalInput").ap()
    w_v_d = nc.dram_tensor("w_v", [C, 128], BF16, kind="ExternalInput").ap()
    w_pT_d = nc.dram_tensor("w_projT", [128, C], FP32R, kind="ExternalInput").ap()
    cos_d = nc.dram_tensor("rope_cos", [128, T], FP16, kind="ExternalInput").ap()
    sin_d = nc.dram_tensor("rope_sin", [128, T], FP16, kind="ExternalInput").ap()
    mask_d = nc.dram_tensor("masks", [128, 4 * CHUNK], BF16, kind="ExternalInput").ap()
    out_d = nc.dram_tensor("outT", [C, T], BF16, kind="ExternalOutput").ap()

    with tile.TileContext(nc) as tc:
        with (
            tc.tile_pool(name="persist", bufs=1) as pers,
            tc.tile_pool(name="xin", bufs=2) as xin,
            tc.tile_pool(name="tmp", bufs=3) as tmps,
            tc.tile_pool(name="ptile", bufs=6) as ptile,
            tc.tile_pool(name="ostage", bufs=3) as ostage,
            tc.tile_pool(name="small", bufs=4) as small,
            tc.tile_pool(name="psUW", bufs=2, space="PSUM") as psUW,  # qkv accum + v transposes
            tc.tile_pool(name="psS", bufs=2, space="PSUM") as psS,    # paired score tiles
            tc.tile_pool(name="psY", bufs=1, space="PSUM") as psY,    # y accum
            tc.tile_pool(name="psX", bufs=1, space="PSUM") as psX,    # warm + out proj
        ):
            # ---- persistent SBUF ----
            wqk_u = pers.tile([128, 6, 256], BF16)
            wqk_w = pers.tile([128, 6, 256], BF16)
            w_v = pers.tile([128, 6, 128], BF16)
            w_pT = pers.tile([128, C], FP32R)

            mask_sb = pers.tile([128, 4 * CHUNK], BF16)
            QT = pers.tile([128, T], BF16)    # rows 0-63 head A, 64-127 head B
            KTt = pers.tile([128, T], BF16)
            V = pers.tile([128, 32, 130], BF16)  # [key%128, keytile, vA|1|vB|1]
            Y = pers.tile([128, T], FP32R)    # normalized attention out [ych, q]
            ones_sb = pers.tile([128, D], BF16)
            ident = pers.tile([128, 128], BF16)
            make_identity(nc, ident[:])

            # only wqk_u is on the first-matmul critical path; the other
            # weight loads are emitted inside emit_qkv(0) after the first
            # x-chunk DMA so they don't delay it
            nc.sync.dma_start(wqk_u[:], wqk_u_d.rearrange("(o p) m -> p o m", p=128))
            ones_f32 = pers.tile([128, D], FP32)
            nc.any.memset(ones_f32[:], 1.0)
            # HAM warm-up: keep PE busy during the initial input DMAs so the
            # clock gate reaches 8/8 before the first real matmuls (results
            # discarded; the tiny copy keeps DCE from dropping the chain)
            warm_ps = psX.tile([128, 128], FP32, tag="o")
            for i in range(16):
                nc.tensor.matmul(warm_ps[:], ident[:, 0:128], ident[:, 0:128],
                                 start=True, stop=True)
            warm_sb = small.tile([1, 8], FP32, tag="warm")
            nc.vector.tensor_copy(warm_sb[:], warm_ps[0:1, 0:8])
            # Pool for the small setup copies -- DVE's early queue is the
            # chunk-0 RoPE critical path
            nc.gpsimd.tensor_copy(ones_sb[:], ones_f32[:])
            nc.gpsimd.tensor_copy(V[:, :, 64], ones_sb[:, 0:32])
            nc.gpsimd.tensor_copy(V[:, :, 129], ones_sb[:, 0:32])
            # dummy Pool work before make_identity's affine_select: delays the
            # HAM warm-up to land flush against the first QKV matmuls, keeping
            # the PE power-state ramp inside the initial DMA window
            nc.gpsimd.memset(Y[64:128, :].bitcast(FP32), 0.0)


            def emit_qkv(J):
                cols = slice(J * CHUNK, (J + 1) * CHUNK)
                xt = xin.tile([128, 6, CHUNK], BF16)
                xT_r = xT_d.rearrange("(o p) n -> p o n", p=128)
                nc.sync.dma_start(xt[:, 0:3, :], xT_r[:, 0:3, cols])
                nc.sync.dma_start(xt[:, 3:6, :], xT_r[:, 3:6, cols])
                cs_sb = tmps.tile([128, CHUNK], FP16, tag="cs")
                sn_sb = tmps.tile([128, CHUNK], FP16, tag="sn")
                nc.sync.dma_start(cs_sb[:], cos_d[:, cols])
                nc.sync.dma_start(sn_sb[:], sin_d[:, cols])
                if J == 0:
                    nc.sync.dma_start(
                        wqk_w[:], wqk_w_d.rearrange("(o p) m -> p o m", p=128))
                    nc.gpsimd.dma_start(
                        w_v[:], w_v_d.rearrange("(o p) m -> p o m", p=128))
                    nc.gpsimd.dma_start(mask_sb[:], mask_d[:])
                    nc.gpsimd.dma_start(w_pT[:], w_pT_d[:])

                for qk, tgt in ((0, QT), (1, KTt)):
                    u_ps = psUW.tile([128, CHUNK], FP32, tag="uw")
                    w_ps = psUW.tile([128, CHUNK], FP32, tag="uw")
                    wcol = slice(qk * 128, qk * 128 + 128)
                    for k in range(6):
                        nc.tensor.matmul(u_ps[:], wqk_u[:, k, wcol], xt[:, k, :],
                                         start=(k == 0), stop=(k == 5))
                    for k in range(6):
                        nc.tensor.matmul(w_ps[:], wqk_w[:, k, wcol], xt[:, k, :],
                                         start=(k == 0), stop=(k == 5))
                    tm = tmps.tile([128, CHUNK], BF16, tag="ropetmp")
                    nc.vector.tensor_tensor(tgt[:, cols], u_ps[:], cs_sb[:], ALU.mult)
                    nc.vector.tensor_tensor(tm[:], w_ps[:], sn_sb[:], ALU.mult)
                    nc.vector.tensor_tensor(tgt[:, cols], tgt[:, cols], tm[:], ALU.add)
                # v in [ch, seq] (N=512 full-rate), then PE-transpose per key tile
                v_ps = psUW.tile([128, CHUNK], FP32, tag="uw")
                for k in range(6):
                    nc.tensor.matmul(v_ps[:], w_v[:, k, :], xt[:, k, :],
                                     start=(k == 0), stop=(k == 5))
                vT_sb = tmps.tile([128, CHUNK], BF16, tag="vt")
                nc.vector.tensor_copy(vT_sb[:], v_ps[:])
                for s in range(4):
                    kt_idx = 4 * J + s
                    tr_ps = psUW.tile([128, 128], BF16, tag="uw")
                    nc.tensor.transpose(tr_ps[:], vT_sb[:, s * 128:(s + 1) * 128], ident)
                    nc.vector.tensor_copy(V[:, kt_idx, 0:64], tr_ps[:, 0:64])
                    nc.vector.tensor_copy(V[:, kt_idx, 65:129], tr_ps[:, 64:128])

            def emit_att(J, heads):
                cols = slice(J * CHUNK, (J + 1) * CHUNK)
                nkt = 4 * J + 4
                for h in heads:
                    hsl = slice(64 * h, 64 * h + 64)
                    vsl = slice(65 * h, 65 * h + 65)
                    y_ps = psY.tile([65, CHUNK], FP32, tag="y")
                    for g in range(nkt // 2):
                        s_ps = psS.tile([128, 2, CHUNK], FP32, tag="s")
                        # both tiles of the pair compute from the pair's first
                        # valid query column so one exp instruction can cover
                        # the whole pair; the second tile's columns below its
                        # own causal start are computed-but-never-read
                        q0 = max(0, 128 * (2 * g - 4 * J))
                        for i in range(2):
                            t = 2 * g + i
                            qsub = slice(J * CHUNK + q0, (J + 1) * CHUNK)
                            nc.tensor.matmul(
                                s_ps[:, i, q0:CHUNK],
                                KTt[hsl, t * KT:(t + 1) * KT], QT[hsl, qsub],
                                start=True, stop=True)
                        p_sb = ptile.tile([128, 2, CHUNK], BF16, tag="p")
                        nc.scalar.activation(p_sb[:, :, q0:CHUNK], s_ps[:, :, q0:CHUNK],
                                             AF.Exp, scale=0.125)
                        for i in range(2):
                            t = 2 * g + i
                            d = t - 4 * J
                            qlo = max(0, 128 * d)
                            if d >= 0:
                                # only qq in [qlo, qlo+128) straddles the causal
                                # boundary; columns beyond are fully valid
                                msub = slice(qlo, qlo + KT)
                                nc.gpsimd.tensor_tensor(
                                    p_sb[:, i, msub], p_sb[:, i, msub],
                                    mask_sb[:, d * CHUNK + qlo:d * CHUNK + qlo + KT],
                                    ALU.mult)
                            sub = slice(qlo, CHUNK)
                            nc.tensor.matmul(
                                y_ps[:, sub], V[:, t, vsl], p_sb[:, i, sub],
                                start=(t == 0), stop=(t == nkt - 1))
                    rc = small.tile([1, CHUNK], FP32R, tag="rc")
                    with nc.allow_low_precision(reason="f32r recip for softmax denom"):
                        nc.vector.reciprocal(rc[0:1, :], y_ps[64:65, :])
                    rb = small.tile([64, CHUNK], FP32R, tag="rb")
                    nc.gpsimd.partition_broadcast(rb[:], rc[0:1, :])
                    nc.vector.tensor_tensor(Y[hsl, cols], y_ps[0:64, :], rb[:], ALU.mult)
                # chunks where this core skips the slot-B head contract over
                # head A's 64 Y-rows only (the partner core owns the rest);
                # matmul cost is set by the moving dim, so this is free
                ymax = 128 if 1 in heads else 64
                for m in range(6):
                    # last chunk: QKV is done, so the psUW slots are free --
                    # borrow them to triple-buffer the projection tail
                    otag = "uw" if J == NCHUNK - 1 and m % 2 else "o"
                    o_ps = (psUW if otag == "uw" else psX).tile(
                        [128, CHUNK], FP32, tag=otag)
                    nc.tensor.matmul(o_ps[:], w_pT[0:ymax, m * 128:(m + 1) * 128],
                                     Y[0:ymax, cols], start=True, stop=True)
                    o_sb = ostage.tile([128, CHUNK], BF16, tag="osb")
                    if J == NCHUNK - 1 and m % 2:
                        # ScalarE is out of exp work by the last chunk's
                        # projection -- split the staging copies across engines
                        nc.scalar.copy(o_sb[:], o_ps[:])
                    else:
                        nc.vector.tensor_copy(o_sb[:], o_ps[:])
                    nc.sync.dma_start(out_d[m * 128:(m + 1) * 128, cols], o_sb[:])

            def emit_body(bchunks):
                # attention J emitted right after its QKV; later QKV fills
                # PE idle
                for J in range(NCHUNK):
                    emit_qkv(J)
                    emit_att(J, [0, 1] if J in bchunks else [0])

            pid = nc.partition_id()
            with tc.If(pid < 4) as cmp:
                emit_body(BCHUNKS_LO)
            with cmp.Else():
                emit_body(BCHUNKS_HI)

    nc.compile()
    return nc


def _rope_tables():
    theta = 1.0 / (ROPE_BASE ** (np.arange(0, D, 2, dtype=np.float32) / D))  # [32]
    freqs = np.arange(T, dtype=np.float32)[None, :] * theta[:, None]  # [32, T]
    cos32 = np.cos(freqs).astype(np.float32)
    sin32 = np.sin(freqs).astype(np.float32)
    cos128 = np.tile(cos32, (4, 1)).astype(np.float16)
    sin128 = np.concatenate([-sin32, sin32, -sin32, sin32], axis=0).astype(np.float16)
    return cos128, sin128


def _masks():
    import ml_dtypes
    m = np.zeros((128, 4 * CHUNK), dtype=np.float32)
    kk = np.arange(128)[:, None]
    qq = np.arange(CHUNK)[None, :]
    for d in range(4):
        m[:, d * CHUNK:(d + 1) * CHUNK] = (128 * d + kk <= qq).astype(np.float32)
    return m.astype(ml_dtypes.bfloat16)


def _swap_halves(w):
    # w: [rows multiple of 64, C]; swap 32-row halves within each 64 block
    r = w.reshape(-1, 2, 32, w.shape[-1])
    return r[:, ::-1].reshape(w.shape)


def make_in_maps(x, w_attn, w_proj):
    import ml_dtypes
    xT = np.ascontiguousarray(x.reshape(T, C).T).astype(ml_dtypes.bfloat16)  # [C, T]
    cos128, sin128 = _rope_tables()
    masks = _masks()
    in_maps = []
    for c in range(N_CORES):
        qk_rows = []   # rows of w_attn for [qA, qB, kA, kB]
        v_rows = []    # [vA, vB]
        p_cols = []    # w_proj columns for [A(64), B(64)]
        sel = HEAD_MAP[c]
        for part_base in (0, C):  # q rows then k rows
            for h in sel:
                if h is None:
                    qk_rows.append(np.zeros((64, C), np.float32))
                else:
                    qk_rows.append(w_attn[part_base + 64 * h: part_base + 64 * h + 64])
        for h in sel:
            if h is None:
                v_rows.append(np.zeros((64, C), np.float32))
                p_cols.append(np.zeros((C, 64), np.float32))
            else:
                v_rows.append(w_attn[2 * C + 64 * h: 2 * C + 64 * h + 64])
                p_cols.append(w_proj[:, 64 * h: 64 * h + 64])
        qk = np.concatenate(qk_rows, axis=0)          # [256, C]
        wqk_u = np.ascontiguousarray(qk.T).astype(ml_dtypes.bfloat16)  # [C, 256]
        wqk_w = np.ascontiguousarray(_swap_halves(qk).T).astype(ml_dtypes.bfloat16)
        w_v = np.ascontiguousarray(
            np.concatenate(v_rows, axis=0).T).astype(ml_dtypes.bfloat16)  # [C, 128]
        w_pT = np.ascontiguousarray(np.concatenate(p_cols, axis=1).T)  # [128, C]
        in_maps.append({
            "xT": xT, "wqk_u": wqk_u, "wqk_w": wqk_w, "w_v": w_v,
            "w_projT": w_pT, "rope_cos": cos128, "rope_sin": sin128,
            "masks": masks,
        })
    return in_maps


def kernel(x, w_attn, w_proj):
    global _PROG
    x = np.asarray(x, dtype=np.float32)
    w_attn = np.asarray(w_attn, dtype=np.float32)
    w_proj = np.asarray(w_proj, dtype=np.float32)
    if _PROG is None:
        _PROG = build_program()
    nc = _PROG
    in_maps = make_in_maps(x, w_attn, w_proj)
    res = bass_utils.run_bass_kernel_spmd(nc, in_maps, core_ids=list(range(N_CORES)))
    acc = np.zeros((C, T), dtype=np.float64)
    for c in range(N_CORES):
        acc += res.results[c]["outT"].astype(np.float64)
    return np.ascontiguousarray(acc.T.astype(np.float32)).reshape(1, T, C)


if __name__ == "__main__":
    rng = np.random.default_rng(0)
    x = rng.standard_normal((1, T, C)).astype(np.float32)
    wa = (rng.standard_normal((3 * C, C)) * 0.02).astype(np.float32)
    wp = (rng.standard_normal((C, C)) * 0.02).astype(np.float32)
    y = kernel(x, wa, wp)
    print("kernel out", y.shape, y.dtype, float(np.abs(y).max()))
